# revision 22
# baseline (speedup 1.0000x reference)
"""Cross-attention kernel for 8 TRN2 NeuronCores.

Reference shapes: x [4, 2048, 1024], embeds [4, 2048, 1024],
Wq/Wk/Wv [1024, 1024] (+bias), Wo [1024, 1024] (+bias), H=16 heads, D=64.

Sharding: core c handles batch b = c//2 and head group hg = c%2 (8 heads,
attn-dim slice of 512).  Each core computes a partial output
outT_c [1024, 2048] (fp16); the host sums the two partials per batch
(row-parallel Wo all-reduce done at unshard time); bo is folded into the
even core's partial.

All matmul operands are fp16 (PSUM accumulates fp32).  Device dataflow:
  QT[m] = Wq_m^T @ xT      [128, 2048] per ad-tile m (4)   feature-major
  KT[m] = Wk_m^T @ embT    [128, 2048]
  V[t]  = embT_t^T @ Wv    [128, 520]  token-major, 8 heads x (64 cols + ones)
  per head h, q-half qh (1024 q):
    per lk-tile t: S = K_h-slice^T-form @ Q_h -> psum [128 lk, 1024 q]
                   E = exp(S/8)               -> sbuf fp16 (ACT, 1024-wide)
                   Cu[qc] += E_chunk^T @ [V_h|1]   psum [128 q, 65] per qc
    normalize: ctx_tok = Cu[:, :64] / Cu[:, 64]  (DVE divide, per-q scalar)
  transpose ctx_tok [q, ad] -> CT [ad, q] via XBAR DMA transpose (fp16)
  outT = Wo^T @ CT  + bo (even core)    -> fp16 out
Softmax skips the max-subtraction: scores ~ N(0,1), exp is safe in fp32.

Scheduling: engines execute their queues in order, so emission order is the
schedule.  The whole kernel is one "tick" loop over the 256 exp tiles
(qh, head, lk-tile): each tick emits the tile's S matmuls + exp, the ctx
matmuls of the tile two back (E-ring), any projection units whose deadline
arrived, and a budgeted trickle of remaining projection / output work.
This keeps the activation engine's exp stream (the ~266us floor) running
back-to-back while the PE (the ~274us floor) stays saturated.  The ctx
matmul is token-major (65-wide moving operand) because a matmul costs its
output free size: 8 heads x 16 q-chunks x 16 lk-passes x 65 halves the PE
cost vs the feature-major form.
"""

import sys

if "/opt/trn_rl_repo" not in sys.path:
    sys.path.insert(0, "/opt/trn_rl_repo")

import numpy as np

import concourse.bass as bass  # noqa: F401
import concourse.mybir as mybir
import concourse.tile as tile
from concourse import bacc
from concourse.bass_utils import run_bass_kernel_spmd

P = 128
B, LQ, LK, DIM = 4, 2048, 2048, 1024
H, D = 16, 64
ADC = 512          # per-core attention dim (8 heads x 64)
NHC = 8            # heads per core
SCALE = 1.0 / 8.0
F32 = mybir.dt.float32
F16 = mybir.dt.float16
EXP = mybir.ActivationFunctionType.Exp

K_T = DIM // P     # 8 contraction tiles for projections
M_AD = ADC // P    # 4 ad partition tiles (head pairs)
T_LK = LK // P     # 16 lk tiles
VW = D + 1         # 65: per-head V block width (64 cols + ones col)
VTW = NHC * VW     # 520: V block width per lk tile
LAG = 3            # ctx matmuls trail exp by this many ticks
ERING = 12         # E-ring depth (sbuf fp16 [128, 1024] slots)

_CACHE = {}


def _build():
    nc = bacc.Bacc("TRN2", target_bir_lowering=False, debug=False)

    xT = nc.dram_tensor("xT", [DIM, LQ], F16, kind="ExternalInput").ap()
    embT = nc.dram_tensor("embT", [DIM, LK], F16, kind="ExternalInput").ap()
    Wq = nc.dram_tensor("Wq", [DIM, ADC], F16, kind="ExternalInput").ap()
    Wk = nc.dram_tensor("Wk", [DIM, ADC], F16, kind="ExternalInput").ap()
    Wv = nc.dram_tensor("Wv", [DIM, ADC], F16, kind="ExternalInput").ap()
    Wo = nc.dram_tensor("Wo", [ADC, DIM], F16, kind="ExternalInput").ap()
    bq = nc.dram_tensor("bq", [P, M_AD], F32, kind="ExternalInput").ap()
    bk = nc.dram_tensor("bk", [P, M_AD], F32, kind="ExternalInput").ap()
    bvb = nc.dram_tensor("bvb", [P, ADC], F32, kind="ExternalInput").ap()
    bo = nc.dram_tensor("bo", [P, DIM // P], F32, kind="ExternalInput").ap()
    ident = nc.dram_tensor("ident", [P, P], F16, kind="ExternalInput").ap()
    outT = nc.dram_tensor("outT", [DIM, LQ], F16, kind="ExternalOutput").ap()

    with tile.TileContext(nc) as tc:
        with tc.tile_pool(name="resident", bufs=1) as res:
            xs = res.tile([P, K_T, LQ], F16, name="xs")
            es = res.tile([P, K_T, LK], F16, name="es")
            wq_sb = res.tile([P, K_T, ADC], F16, name="wq")
            wk_sb = res.tile([P, K_T, ADC], F16, name="wk")
            wv_sb = res.tile([P, K_T, ADC], F16, name="wv")
            wo_sb = res.tile([P, M_AD, DIM], F16, name="wo")
            QT = [res.tile([P, LQ], F16, name=f"qt{m}") for m in range(M_AD)]
            KT = [res.tile([P, LK], F16, name=f"kt{m}") for m in range(M_AD)]
            V = res.tile([P, T_LK * VTW], F16, name="v")
            CT = [[res.tile([P, LQ // 2], F16, name=f"ct{m}_{q}")
                   for q in range(2)] for m in range(M_AD)]
            bq_sb = res.tile([P, M_AD], F32, name="bq")
            bk_sb = res.tile([P, M_AD], F32, name="bk")
            bvb_sb = res.tile([P, ADC], F32, name="bvb")
            bo_sb = res.tile([P, DIM // P], F32, name="bo")
            id_sb = res.tile([P, P], F16, name="ident")

            # ---- input DMAs (SP queue; loads only, never block) ----
            # Ordered so the first S matmul (needs K pair 0 chunk 0 + Q pair
            # 0 q-half 0) can launch as early as possible.
            embT_kp = embT.rearrange("(k p) n -> p k n", p=P)
            xT_kp = xT.rearrange("(k p) n -> p k n", p=P)
            nc.sync.dma_start(wk_sb[:], Wk.rearrange("(k p) n -> p k n", p=P))
            nc.sync.dma_start(es[:, :, 0:512], embT_kp[:, :, 0:512])
            nc.sync.dma_start(bk_sb[:], bk[:])
            nc.sync.dma_start(wq_sb[:], Wq.rearrange("(k p) n -> p k n", p=P))
            nc.sync.dma_start(xs[:, :, 0:512], xT_kp[:, :, 0:512])
            nc.sync.dma_start(bq_sb[:], bq[:])
            nc.sync.dma_start(xs[:, :, 512:1024], xT_kp[:, :, 512:1024])
            nc.sync.dma_start(wv_sb[:], Wv.rearrange("(k p) n -> p k n", p=P))
            nc.sync.dma_start(bvb_sb[:], bvb[:])
            nc.sync.dma_start(id_sb[:], ident[:])
            for n in range(1, 4):
                nc.sync.dma_start(es[:, :, n * 512:(n + 1) * 512],
                                  embT_kp[:, :, n * 512:(n + 1) * 512])
            nc.sync.dma_start(xs[:, :, 1024:2048], xT_kp[:, :, 1024:2048])
            nc.sync.dma_start(wo_sb[:], Wo.rearrange("(k p) n -> p k n", p=P))
            nc.sync.dma_start(bo_sb[:], bo[:])

            # ones columns for the fused-denominator ctx matmul: preset the
            # whole V tile to 1.0; V-proj bias-add overwrites the 64-wide
            # value blocks and leaves column 64 of each head block intact.
            nc.gpsimd.memset(V[:], 1.0)

            with tc.tile_pool(name="pj", bufs=2, space="PSUM") as pjp, \
                 tc.tile_pool(name="sw", bufs=2, space="PSUM") as swp, \
                 tc.tile_pool(name="cp", bufs=1, space="PSUM") as cpp, \
                 tc.tile_pool(name="ep", bufs=ERING) as epp, \
                 tc.tile_pool(name="ctok", bufs=2) as ctokp, \
                 tc.tile_pool(name="rcp", bufs=2) as rcpp, \
                 tc.tile_pool(name="os", bufs=4) as osp:

                # ---------- emission helpers ----------
                def emit_kproj(m, n):
                    ps = pjp.tile([P, 512], F32, name="pp")
                    for k in range(K_T):
                        nc.tensor.matmul(
                            ps[:], wk_sb[:, k, m * P:(m + 1) * P],
                            es[:, k, n * 512:(n + 1) * 512],
                            start=(k == 0), stop=(k == K_T - 1))
                    nc.vector.tensor_scalar_add(
                        KT[m][:, n * 512:(n + 1) * 512], ps[:],
                        bk_sb[:, m:m + 1])

                def emit_qproj(m, n):
                    ps = pjp.tile([P, 512], F32, name="pp")
                    for k in range(K_T):
                        nc.tensor.matmul(
                            ps[:], wq_sb[:, k, m * P:(m + 1) * P],
                            xs[:, k, n * 512:(n + 1) * 512],
                            start=(k == 0), stop=(k == K_T - 1))
                    nc.vector.tensor_scalar_add(
                        QT[m][:, n * 512:(n + 1) * 512], ps[:],
                        bq_sb[:, m:m + 1])

                def emit_vproj(t):
                    ps = pjp.tile([P, 512], F32, name="pp")
                    for k in range(K_T):
                        nc.tensor.matmul(
                            ps[:], es[:, k, t * P:(t + 1) * P],
                            wv_sb[:, k, :],
                            start=(k == 0), stop=(k == K_T - 1))
                    vdst = V[:, t * VTW:(t + 1) * VTW].rearrange(
                        "p (a b) -> p a b", b=VW)
                    nc.vector.tensor_tensor(
                        vdst[:, :, 0:D],
                        ps[:].rearrange("p (a b) -> p a b", b=D),
                        bvb_sb[:].rearrange("p (a b) -> p a b", b=D),
                        op=mybir.AluOpType.add)

                def emit_outproj(d, qn):
                    po = pjp.tile([P, 512], F32, name="pp")
                    for ch in range(M_AD):
                        nc.tensor.matmul(
                            po[:],
                            wo_sb[:, ch, d * P:(d + 1) * P],
                            CT[ch][qn // 2][:, (qn % 2) * 512:
                                            (qn % 2) * 512 + 512],
                            start=(ch == 0), stop=(ch == M_AD - 1))
                    ot = osp.tile([P, 512], F16, name="ot")
                    nc.vector.tensor_scalar_add(ot[:], po[:],
                                                bo_sb[:, d:d + 1])
                    nc.sync.dma_start(
                        outT[d * P:(d + 1) * P, qn * 512:(qn + 1) * 512],
                        ot[:])

                # ---------- deferred work with deadlines ----------
                # Units: (deadline_tick, avail_tick, est_ns, fn).  Forced
                # when their deadline tick arrives (just before that tick's
                # S matmuls need the result); otherwise trickled in whenever
                # the emitted-PE-work clock lags the dynamically tracked ACT
                # clock, so the exp stream is never starved by front-loaded
                # PE work and the PE never runs dry while exps stream.
                PROJ_NS = 1830   # 8 passes x 512 cols (+overheads)
                OUT_NS = 930     # 4 passes x 512 cols
                S_NS = 470
                C_NS = 240
                EXP_NS = 1040
                LAT = 500.0      # S-done -> exp-start latency
                work = []
                vdone = [False] * T_LK

                def vproj_unit(t):
                    if not vdone[t]:
                        vdone[t] = True
                        emit_vproj(t)
                        return PROJ_NS
                    return 0.0

                for t in range(T_LK):
                    avail = (0 if t < 2 else (2 if t < 4 else
                             (4 if t < 8 else (7 if t < 12 else 10))))
                    work.append((t + ERING - 3, avail, PROJ_NS,
                                 lambda t=t: vproj_unit(t)))
                for m in range(1, M_AD):
                    t0 = 32 * m
                    work.append((t0 - 5, 0, PROJ_NS,
                                 lambda m=m: emit_qproj(m, 0)))
                    work.append((t0 - 4, 0, PROJ_NS,
                                 lambda m=m: emit_qproj(m, 1)))
                    for n in range(4):
                        work.append((t0 + 4 * n - 4, 0, PROJ_NS,
                                     lambda m=m, n=n: emit_kproj(m, n)))
                for m in range(M_AD):
                    t0 = 128 + 32 * m
                    work.append((t0 - 9, 16, PROJ_NS,
                                 lambda m=m: emit_qproj(m, 2)))
                    work.append((t0 - 8, 16, PROJ_NS,
                                 lambda m=m: emit_qproj(m, 3)))
                for i, d in enumerate(range(DIM // P)):
                    for qn in range(2):
                        work.append((250, 134 + 3 * i, OUT_NS,
                                     lambda d=d, qn=qn: emit_outproj(d, qn)))
                work.sort(key=lambda w: w[0])

                clk = {"pe": 0.0, "act": 0.0}

                def run_unit(i):
                    _, _, ns, fn = work.pop(i)
                    r = fn()
                    clk["pe"] += ns if r is None else r

                # ---------- startup (DMA-window) projections ----------
                # estimated DMA completion: wk 3.3, es-n0 6.2, wv 9.4,
                # wq 12.2, xs-n0 15.2, xs-n1 18.2 (us, incl. latency)
                clk["pe"] = 7100.0
                emit_kproj(0, 0)
                clk["pe"] += PROJ_NS
                clk["pe"] = max(clk["pe"], 13000.0)
                emit_qproj(0, 0)
                clk["pe"] += PROJ_NS
                clk["act"] = clk["pe"] + 1283.0  # act table load

                # ---------- global tick loop ----------
                ticks = [(qh, m, hh, t)
                         for qh in range(2)
                         for m in range(M_AD)
                         for hh in range(2)
                         for t in range(T_LK)]
                ering = [None] * ERING
                head_state = {}  # head index (g // T_LK) -> state dict
                cur = {"ctok": None, "qh": -1}

                def cuv(cu, qc):
                    off = (qc // 4) * 512 + (qc % 4) * VW
                    return cu[:, off:off + VW]

                def emit_ctx(gc):
                    # ctx matmuls for global tile gc; head 0 lazily emits the
                    # V projection for the lk-tile it is about to consume
                    hs = head_state[gc // T_LK]
                    t = gc % T_LK
                    if gc // T_LK == 0:
                        clk["pe"] += vproj_unit(t)
                    if hs["cu"] is None:
                        hs["cu"] = cpp.tile([P, 1024], F32, name="cu")
                    et = ering[gc % ERING]
                    cu = hs["cu"]
                    voff = t * VTW + hs["h"] * VW
                    for qc in range(8):
                        # start=True lazily zeroes the whole 2KB psum bank,
                        # so only the first matmul touching each bank sets it
                        nc.tensor.matmul(
                            cuv(cu, qc),
                            et[:, qc * P:(qc + 1) * P],
                            V[:, voff:voff + VW],
                            start=(t == 0 and qc % 4 == 0),
                            stop=(t == T_LK - 1))
                    clk["pe"] += C_NS
                    if t == T_LK - 1:
                        finish_head(hs)
                        del head_state[gc // T_LK]

                def finish_head(hs):
                    # normalize into ctok; at pair end, XBAR-transpose
                    cu, ctok, h = hs["cu"], hs["ctok"], hs["h"]
                    rcp = rcpp.tile([P, 8], F32, name="rcp")
                    for half in range(2):
                        dn = cu[:, half * 512:half * 512 + 4 * VW].rearrange(
                            "p (a b) -> p a b", b=VW)[:, :, D:D + 1]
                        nc.vector.reciprocal(
                            rcp[:, half * 4:(half + 1) * 4].rearrange(
                                "p (a b) -> p a b", b=1), dn)
                    for qc in range(8):
                        nc.vector.tensor_scalar(
                            ctok[:, qc, h * D:(h + 1) * D],
                            cuv(cu, qc)[:, 0:D],
                            rcp[:, qc:qc + 1], None,
                            op0=mybir.AluOpType.mult)
                    if hs["hh"] == 1:
                        m, q0 = hs["m"], hs["q0"]
                        qi = q0 // 1024
                        if m == M_AD - 1 and qi == 1:
                            # final pair: XBAR DMA + HWDGE serialization
                            # (8 x 625ns) would sit on the critical tail;
                            # transpose on the PE instead (8 x 128 cols)
                            # into a spare S-window bank, one DVE copy out
                            tp = swp.tile([P, 2048], F16, name="sw")
                            for qc in range(8):
                                nc.tensor.transpose(
                                    tp[:, qc * P:(qc + 1) * P],
                                    ctok[:, qc, m * P:(m + 1) * P],
                                    id_sb[:])
                            nc.vector.tensor_copy(
                                CT[m][qi][:, 0:1024], tp[:, 0:1024])
                        else:
                            for qc in range(8):
                                nc.sync.dma_start_transpose(
                                    CT[m][qi][:, qc * P:(qc + 1) * P],
                                    ctok[:, qc, m * P:(m + 1) * P])

                cnext = [0]  # next global tile whose ctx matmuls are pending

                def cmax(g):
                    # elastic ctx lag: a fresh head's first tiles trail by 5
                    # ticks so its cu-bank WAR on the previous head's
                    # normalize (DVE) is off the PE's critical path
                    return g - (5 if cnext[0] % T_LK < 2 else 1)

                for g, (qh, m, hh, t) in enumerate(ticks):
                    if t == 0:
                        if qh != cur["qh"]:
                            cur["qh"] = qh
                            cur["ctok"] = ctokp.tile(
                                [P, LQ // P // 2, ADC], F16, name="ctok")
                        head_state[g // T_LK] = {
                            "h": 2 * m + hh, "m": m, "hh": hh,
                            "q0": qh * 1024, "gbase": g,
                            "cu": None, "ctok": cur["ctok"],
                        }
                    ro = hh * D
                    q0 = qh * 1024

                    # mandatory: deadlines, pair-0 K chunks, E-ring pressure
                    while work and work[0][0] <= g:
                        run_unit(0)
                    if m == 0 and hh == 0 and qh == 0 and t in (4, 8, 12):
                        emit_kproj(0, t // 4)
                        clk["pe"] = max(clk["pe"],
                                        19100.0 + (t // 4 - 1) * 2900.0) \
                            + PROJ_NS
                    while cnext[0] <= g - ERING + 2:
                        emit_ctx(cnext[0])
                        cnext[0] += 1

                    # paced work: fill the PE up to the point where this
                    # tick's S matmuls still land before exp(g-1) ends
                    budget = clk["act"] - LAT - S_NS
                    progress = True
                    while progress:
                        progress = False
                        if cnext[0] <= cmax(g) and \
                                clk["pe"] + C_NS <= budget:
                            emit_ctx(cnext[0])
                            cnext[0] += 1
                            progress = True
                            continue
                        for i in range(len(work)):
                            if work[i][1] <= g and \
                                    clk["pe"] + work[i][2] <= budget:
                                run_unit(i)
                                progress = True
                                break

                    # S matmuls for this tick's lk-tile, then exp.
                    # Tick 0 goes in 512-wide halves so the first exp can
                    # start before the second q-chunk's Q projection is done.
                    sw = swp.tile([P, 1024], F32, name="sw")
                    et = epp.tile([P, 1024], F16, name="et")
                    if g == 0:
                        nc.tensor.matmul(
                            sw[:, 0:512], KT[0][0:D, 0:P],
                            QT[0][0:D, 0:512], start=True, stop=True)
                        nc.scalar.activation(et[:, 0:512], sw[:, 0:512],
                                             EXP, scale=SCALE)
                        clk["pe"] += S_NS / 2
                        clk["act"] = clk["pe"] + 1283.0 + 612.0
                        clk["pe"] = max(clk["pe"], 15800.0)
                        emit_qproj(0, 1)
                        clk["pe"] += PROJ_NS
                        nc.tensor.matmul(
                            sw[:, 512:1024], KT[0][0:D, 0:P],
                            QT[0][0:D, 512:1024], start=True, stop=True)
                        nc.scalar.activation(et[:, 512:1024],
                                             sw[:, 512:1024],
                                             EXP, scale=SCALE)
                        clk["pe"] += S_NS / 2
                        clk["act"] = max(clk["act"],
                                         clk["pe"] + LAT) + 612.0
                    else:
                        for nn in range(2):
                            nc.tensor.matmul(
                                sw[:, nn * 512:(nn + 1) * 512],
                                KT[m][ro:ro + D, t * P:(t + 1) * P],
                                QT[m][ro:ro + D,
                                      q0 + nn * 512:q0 + (nn + 1) * 512],
                                start=True, stop=True)
                        nc.scalar.activation(et[:], sw[:], EXP, scale=SCALE)
                        clk["pe"] += S_NS
                        clk["act"] = max(clk["act"], clk["pe"] + LAT) + EXP_NS
                    ering[g % ERING] = et

                # ---------- tail ----------
                while cnext[0] < 256:
                    emit_ctx(cnext[0])
                    cnext[0] += 1
                while work:
                    _, _, _, fn = work.pop(0)
                    fn()
                for d in range(DIM // P):
                    emit_outproj(d, 2)
                    emit_outproj(d, 3)

    nc.compile()
    return nc


def _in_maps(x, embeds, Wq, bq, Wk, bk, Wv, bv, Wo, bo):
    h = np.float16
    f = np.float32
    maps = []
    for c in range(8):
        b, hg = c // 2, c % 2
        s = slice(hg * ADC, (hg + 1) * ADC)
        bo_c = bo if hg == 0 else np.zeros_like(bo)
        maps.append({
            "xT": np.ascontiguousarray(x[b].T, dtype=h),
            "embT": np.ascontiguousarray(embeds[b].T, dtype=h),
            "Wq": np.ascontiguousarray(Wq[:, s], dtype=h),
            "Wk": np.ascontiguousarray(Wk[:, s], dtype=h),
            "Wv": np.ascontiguousarray(Wv[:, s], dtype=h),
            "Wo": np.ascontiguousarray(Wo[s, :], dtype=h),
            "bq": np.ascontiguousarray(bq[s].reshape(M_AD, P).T, dtype=f),
            "bk": np.ascontiguousarray(bk[s].reshape(M_AD, P).T, dtype=f),
            "bvb": np.ascontiguousarray(np.tile(bv[s], (P, 1)), dtype=f),
            "bo": np.ascontiguousarray(
                bo_c.reshape(DIM // P, P).T, dtype=f),
            "ident": np.eye(P, dtype=h),
        })
    return maps


def kernel(x, embeds, Wq, bq, Wk, bk, Wv, bv, Wo, bo, _trace=False,
           _tmpdir=None):
    x = np.asarray(x); embeds = np.asarray(embeds)
    Wq = np.asarray(Wq); bq = np.asarray(bq)
    Wk = np.asarray(Wk); bk = np.asarray(bk)
    Wv = np.asarray(Wv); bv = np.asarray(bv)
    Wo = np.asarray(Wo); bo = np.asarray(bo)

    if "nc" not in _CACHE:
        _CACHE["nc"] = _build()
    nc = _CACHE["nc"]

    maps = _in_maps(x, embeds, Wq, bq, Wk, bk, Wv, bv, Wo, bo)
    res = run_bass_kernel_spmd(nc, maps, core_ids=list(range(8)),
                               trace=_trace, tmpdir=_tmpdir)
    if _trace:
        _CACHE["last_exec_time_ns"] = res.exec_time_ns
    _CACHE["last_results"] = res

    out = np.empty((B, LQ, DIM), np.float32)
    for b in range(B):
        acc = (res.results[2 * b]["outT"].astype(np.float32)
               + res.results[2 * b + 1]["outT"].astype(np.float32))
        out[b] = acc.T
    return out


# revision 23
# speedup vs baseline: 1.0037x; 1.0037x over previous
"""Cross-attention kernel for 8 TRN2 NeuronCores.

Reference shapes: x [4, 2048, 1024], embeds [4, 2048, 1024],
Wq/Wk/Wv [1024, 1024] (+bias), Wo [1024, 1024] (+bias), H=16 heads, D=64.

Sharding: core c handles batch b = c//2 and head group hg = c%2 (8 heads,
attn-dim slice of 512).  Each core computes a partial output
outT_c [1024, 2048] (fp16); the host sums the two partials per batch
(row-parallel Wo all-reduce done at unshard time); bo is folded into the
even core's partial.

All matmul operands are fp16 (PSUM accumulates fp32).  Device dataflow:
  QT[m] = Wq_m^T @ xT      [128, 2048] per ad-tile m (4)   feature-major
  KT[m] = Wk_m^T @ embT    [128, 2048]
  V[t]  = embT_t^T @ Wv    [128, 520]  token-major, 8 heads x (64 cols + ones)
  per head h, q-half qh (1024 q):
    per lk-tile t: S = K_h-slice^T-form @ Q_h -> psum [128 lk, 1024 q]
                   E = exp(S/8)               -> sbuf fp16 (ACT, 1024-wide)
                   Cu[qc] += E_chunk^T @ [V_h|1]   psum [128 q, 65] per qc
    normalize: ctx_tok = Cu[:, :64] / Cu[:, 64]  (DVE divide, per-q scalar)
  transpose ctx_tok [q, ad] -> CT [ad, q] via XBAR DMA transpose (fp16)
  outT = Wo^T @ CT  + bo (even core)    -> fp16 out
Softmax skips the max-subtraction: scores ~ N(0,1), exp is safe in fp32.

Scheduling: engines execute their queues in order, so emission order is the
schedule.  The whole kernel is one "tick" loop over the 256 exp tiles
(qh, head, lk-tile): each tick emits the tile's S matmuls + exp, the ctx
matmuls of the tile two back (E-ring), any projection units whose deadline
arrived, and a budgeted trickle of remaining projection / output work.
This keeps the activation engine's exp stream (the ~266us floor) running
back-to-back while the PE (the ~274us floor) stays saturated.  The ctx
matmul is token-major (65-wide moving operand) because a matmul costs its
output free size: 8 heads x 16 q-chunks x 16 lk-passes x 65 halves the PE
cost vs the feature-major form.
"""

import sys

if "/opt/trn_rl_repo" not in sys.path:
    sys.path.insert(0, "/opt/trn_rl_repo")

import numpy as np

import concourse.bass as bass  # noqa: F401
import concourse.mybir as mybir
import concourse.tile as tile
from concourse import bacc
from concourse.bass_utils import run_bass_kernel_spmd

P = 128
B, LQ, LK, DIM = 4, 2048, 2048, 1024
H, D = 16, 64
ADC = 512          # per-core attention dim (8 heads x 64)
NHC = 8            # heads per core
SCALE = 1.0 / 8.0
F32 = mybir.dt.float32
F16 = mybir.dt.float16
EXP = mybir.ActivationFunctionType.Exp

K_T = DIM // P     # 8 contraction tiles for projections
M_AD = ADC // P    # 4 ad partition tiles (head pairs)
T_LK = LK // P     # 16 lk tiles
VW = D + 1         # 65: per-head V block width (64 cols + ones col)
VTW = NHC * VW     # 520: V block width per lk tile
LAG = 3            # ctx matmuls trail exp by this many ticks
ERING = 12         # E-ring depth (sbuf fp16 [128, 1024] slots)

_CACHE = {}


def _build():
    nc = bacc.Bacc("TRN2", target_bir_lowering=False, debug=False)

    xT = nc.dram_tensor("xT", [DIM, LQ], F16, kind="ExternalInput").ap()
    embT = nc.dram_tensor("embT", [DIM, LK], F16, kind="ExternalInput").ap()
    Wq = nc.dram_tensor("Wq", [DIM, ADC], F16, kind="ExternalInput").ap()
    Wk = nc.dram_tensor("Wk", [DIM, ADC], F16, kind="ExternalInput").ap()
    Wv = nc.dram_tensor("Wv", [DIM, ADC], F16, kind="ExternalInput").ap()
    Wo = nc.dram_tensor("Wo", [ADC, DIM], F16, kind="ExternalInput").ap()
    bq = nc.dram_tensor("bq", [P, M_AD], F32, kind="ExternalInput").ap()
    bk = nc.dram_tensor("bk", [P, M_AD], F32, kind="ExternalInput").ap()
    bvb = nc.dram_tensor("bvb", [P, ADC], F32, kind="ExternalInput").ap()
    bo = nc.dram_tensor("bo", [P, DIM // P], F32, kind="ExternalInput").ap()
    ident = nc.dram_tensor("ident", [P, P], F16, kind="ExternalInput").ap()
    outT = nc.dram_tensor("outT", [DIM, LQ], F16, kind="ExternalOutput").ap()

    with tile.TileContext(nc) as tc:
        with tc.tile_pool(name="resident", bufs=1) as res:
            xs = res.tile([P, K_T, LQ], F16, name="xs")
            es = res.tile([P, K_T, LK], F16, name="es")
            wq_sb = res.tile([P, K_T, ADC], F16, name="wq")
            wk_sb = res.tile([P, K_T, ADC], F16, name="wk")
            wv_sb = res.tile([P, K_T, ADC], F16, name="wv")
            wo_sb = res.tile([P, M_AD, DIM], F16, name="wo")
            QT = [res.tile([P, LQ], F16, name=f"qt{m}") for m in range(M_AD)]
            KT = [res.tile([P, LK], F16, name=f"kt{m}") for m in range(M_AD)]
            V = res.tile([P, T_LK * VTW], F16, name="v")
            CT = [[res.tile([P, LQ // 2], F16, name=f"ct{m}_{q}")
                   for q in range(2)] for m in range(M_AD)]
            bq_sb = res.tile([P, M_AD], F32, name="bq")
            bk_sb = res.tile([P, M_AD], F32, name="bk")
            bvb_sb = res.tile([P, ADC], F32, name="bvb")
            bo_sb = res.tile([P, DIM // P], F32, name="bo")
            id_sb = res.tile([P, P], F16, name="ident")

            # ---- input DMAs (SP queue; loads only, never block) ----
            # Ordered so the first S matmul (needs K pair 0 chunk 0 + Q pair
            # 0 q-half 0) can launch as early as possible.
            embT_kp = embT.rearrange("(k p) n -> p k n", p=P)
            xT_kp = xT.rearrange("(k p) n -> p k n", p=P)
            nc.sync.dma_start(wk_sb[:], Wk.rearrange("(k p) n -> p k n", p=P))
            nc.sync.dma_start(es[:, :, 0:512], embT_kp[:, :, 0:512])
            nc.sync.dma_start(bk_sb[:], bk[:])
            nc.sync.dma_start(wq_sb[:], Wq.rearrange("(k p) n -> p k n", p=P))
            nc.sync.dma_start(xs[:, :, 0:512], xT_kp[:, :, 0:512])
            nc.sync.dma_start(bq_sb[:], bq[:])
            nc.sync.dma_start(xs[:, :, 512:1024], xT_kp[:, :, 512:1024])
            nc.sync.dma_start(wv_sb[:], Wv.rearrange("(k p) n -> p k n", p=P))
            nc.sync.dma_start(bvb_sb[:], bvb[:])
            nc.sync.dma_start(id_sb[:], ident[:])
            for n in range(1, 4):
                nc.sync.dma_start(es[:, :, n * 512:(n + 1) * 512],
                                  embT_kp[:, :, n * 512:(n + 1) * 512])
            nc.sync.dma_start(xs[:, :, 1024:2048], xT_kp[:, :, 1024:2048])
            nc.sync.dma_start(wo_sb[:], Wo.rearrange("(k p) n -> p k n", p=P))
            nc.sync.dma_start(bo_sb[:], bo[:])

            # ones columns for the fused-denominator ctx matmul: preset the
            # whole V tile to 1.0; V-proj bias-add overwrites the 64-wide
            # value blocks and leaves column 64 of each head block intact.
            nc.gpsimd.memset(V[:], 1.0)

            with tc.tile_pool(name="pj", bufs=2, space="PSUM") as pjp, \
                 tc.tile_pool(name="sw", bufs=2, space="PSUM") as swp, \
                 tc.tile_pool(name="cp", bufs=1, space="PSUM") as cpp, \
                 tc.tile_pool(name="ep", bufs=ERING) as epp, \
                 tc.tile_pool(name="ctok", bufs=2) as ctokp, \
                 tc.tile_pool(name="rcp", bufs=2) as rcpp, \
                 tc.tile_pool(name="os", bufs=4) as osp:

                # ---------- emission helpers ----------
                def emit_kproj(m, n):
                    ps = pjp.tile([P, 512], F32, name="pp")
                    for k in range(K_T):
                        nc.tensor.matmul(
                            ps[:], wk_sb[:, k, m * P:(m + 1) * P],
                            es[:, k, n * 512:(n + 1) * 512],
                            start=(k == 0), stop=(k == K_T - 1))
                    nc.vector.tensor_scalar_add(
                        KT[m][:, n * 512:(n + 1) * 512], ps[:],
                        bk_sb[:, m:m + 1])

                def emit_qproj(m, n):
                    ps = pjp.tile([P, 512], F32, name="pp")
                    for k in range(K_T):
                        nc.tensor.matmul(
                            ps[:], wq_sb[:, k, m * P:(m + 1) * P],
                            xs[:, k, n * 512:(n + 1) * 512],
                            start=(k == 0), stop=(k == K_T - 1))
                    nc.vector.tensor_scalar_add(
                        QT[m][:, n * 512:(n + 1) * 512], ps[:],
                        bq_sb[:, m:m + 1])

                def emit_vproj(t):
                    ps = pjp.tile([P, 512], F32, name="pp")
                    for k in range(K_T):
                        nc.tensor.matmul(
                            ps[:], es[:, k, t * P:(t + 1) * P],
                            wv_sb[:, k, :],
                            start=(k == 0), stop=(k == K_T - 1))
                    vdst = V[:, t * VTW:(t + 1) * VTW].rearrange(
                        "p (a b) -> p a b", b=VW)
                    nc.vector.tensor_tensor(
                        vdst[:, :, 0:D],
                        ps[:].rearrange("p (a b) -> p a b", b=D),
                        bvb_sb[:].rearrange("p (a b) -> p a b", b=D),
                        op=mybir.AluOpType.add)

                def emit_outproj(d, qn):
                    po = pjp.tile([P, 512], F32, name="pp")
                    for ch in range(M_AD):
                        nc.tensor.matmul(
                            po[:],
                            wo_sb[:, ch, d * P:(d + 1) * P],
                            CT[ch][qn // 2][:, (qn % 2) * 512:
                                            (qn % 2) * 512 + 512],
                            start=(ch == 0), stop=(ch == M_AD - 1))
                    ot = osp.tile([P, 512], F16, name="ot")
                    nc.vector.tensor_scalar_add(ot[:], po[:],
                                                bo_sb[:, d:d + 1])
                    nc.sync.dma_start(
                        outT[d * P:(d + 1) * P, qn * 512:(qn + 1) * 512],
                        ot[:])

                # ---------- deferred work with deadlines ----------
                # Units: (deadline_tick, avail_tick, est_ns, fn).  Forced
                # when their deadline tick arrives (just before that tick's
                # S matmuls need the result); otherwise trickled in whenever
                # the emitted-PE-work clock lags the dynamically tracked ACT
                # clock, so the exp stream is never starved by front-loaded
                # PE work and the PE never runs dry while exps stream.
                PROJ_NS = 1830   # 8 passes x 512 cols (+overheads)
                OUT_NS = 930     # 4 passes x 512 cols
                S_NS = 470
                C_NS = 240
                EXP_NS = 1040
                LAT = 500.0      # S-done -> exp-start latency
                work = []
                vdone = [False] * T_LK

                def vproj_unit(t):
                    if not vdone[t]:
                        vdone[t] = True
                        emit_vproj(t)
                        return PROJ_NS
                    return 0.0

                for t in range(T_LK):
                    avail = (0 if t < 2 else (2 if t < 4 else
                             (4 if t < 8 else (7 if t < 12 else 10))))
                    work.append((t + ERING - 3, avail, PROJ_NS,
                                 lambda t=t: vproj_unit(t)))
                for m in range(1, M_AD):
                    t0 = 32 * m
                    work.append((t0 - 5, 0, PROJ_NS,
                                 lambda m=m: emit_qproj(m, 0)))
                    work.append((t0 - 4, 0, PROJ_NS,
                                 lambda m=m: emit_qproj(m, 1)))
                    for n in range(4):
                        work.append((t0 + 4 * n - 4, 0, PROJ_NS,
                                     lambda m=m, n=n: emit_kproj(m, n)))
                for m in range(M_AD):
                    t0 = 128 + 32 * m
                    work.append((t0 - 9, 16, PROJ_NS,
                                 lambda m=m: emit_qproj(m, 2)))
                    work.append((t0 - 8, 16, PROJ_NS,
                                 lambda m=m: emit_qproj(m, 3)))
                for i, d in enumerate(range(DIM // P)):
                    for qn in range(2):
                        work.append((250, 134 + 3 * i, OUT_NS,
                                     lambda d=d, qn=qn: emit_outproj(d, qn)))
                work.sort(key=lambda w: w[0])

                clk = {"pe": 0.0, "act": 0.0}

                def run_unit(i):
                    _, _, ns, fn = work.pop(i)
                    r = fn()
                    clk["pe"] += ns if r is None else r

                # ---------- startup (DMA-window) projections ----------
                # estimated DMA completion: wk 3.3, es-n0 6.2, wv 9.4,
                # wq 12.2, xs-n0 15.2, xs-n1 18.2 (us, incl. latency)
                clk["pe"] = 7100.0
                emit_kproj(0, 0)
                clk["pe"] += PROJ_NS
                clk["pe"] = max(clk["pe"], 13000.0)
                emit_qproj(0, 0)
                clk["pe"] += PROJ_NS
                clk["act"] = clk["pe"] + 1283.0  # act table load

                # ---------- global tick loop ----------
                ticks = [(qh, m, hh, t)
                         for qh in range(2)
                         for m in range(M_AD)
                         for hh in range(2)
                         for t in range(T_LK)]
                ering = [None] * ERING
                head_state = {}  # head index (g // T_LK) -> state dict
                cur = {"ctok": None, "qh": -1}

                def cuv(cu, qc):
                    off = (qc // 4) * 512 + (qc % 4) * VW
                    return cu[:, off:off + VW]

                def emit_ctx(gc):
                    # ctx matmuls for global tile gc; head 0 lazily emits the
                    # V projection for the lk-tile it is about to consume
                    hs = head_state[gc // T_LK]
                    t = gc % T_LK
                    if gc // T_LK == 0:
                        clk["pe"] += vproj_unit(t)
                    if hs["cu"] is None:
                        hs["cu"] = cpp.tile([P, 1024], F32, name="cu")
                    et = ering[gc % ERING]
                    cu = hs["cu"]
                    voff = t * VTW + hs["h"] * VW
                    for qc in range(8):
                        # start=True lazily zeroes the whole 2KB psum bank,
                        # so only the first matmul touching each bank sets it
                        nc.tensor.matmul(
                            cuv(cu, qc),
                            et[:, qc * P:(qc + 1) * P],
                            V[:, voff:voff + VW],
                            start=(t == 0 and qc % 4 == 0),
                            stop=(t == T_LK - 1))
                    clk["pe"] += C_NS
                    if t == T_LK - 1:
                        finish_head(hs)
                        del head_state[gc // T_LK]

                def finish_head(hs):
                    # normalize into ctok; at pair end, XBAR-transpose
                    cu, ctok, h = hs["cu"], hs["ctok"], hs["h"]
                    rcp = rcpp.tile([P, 8], F32, name="rcp")
                    for half in range(2):
                        dn = cu[:, half * 512:half * 512 + 4 * VW].rearrange(
                            "p (a b) -> p a b", b=VW)[:, :, D:D + 1]
                        nc.vector.reciprocal(
                            rcp[:, half * 4:(half + 1) * 4].rearrange(
                                "p (a b) -> p a b", b=1), dn)
                    for qc in range(8):
                        nc.vector.tensor_scalar(
                            ctok[:, qc, h * D:(h + 1) * D],
                            cuv(cu, qc)[:, 0:D],
                            rcp[:, qc:qc + 1], None,
                            op0=mybir.AluOpType.mult)
                    if hs["hh"] == 1:
                        m, q0 = hs["m"], hs["q0"]
                        qi = q0 // 1024
                        if m == M_AD - 1 and qi == 1:
                            # final pair: XBAR DMA + HWDGE serialization
                            # (8 x 625ns) would sit on the critical tail;
                            # transpose on the PE instead (8 x 128 cols)
                            # into a spare S-window bank, one DVE copy out
                            tp = swp.tile([P, 2048], F16, name="sw")
                            for qc in range(8):
                                nc.tensor.transpose(
                                    tp[:, qc * P:(qc + 1) * P],
                                    ctok[:, qc, m * P:(m + 1) * P],
                                    id_sb[:])
                            nc.vector.tensor_copy(
                                CT[m][qi][:, 0:1024], tp[:, 0:1024])
                        else:
                            for qc in range(8):
                                nc.sync.dma_start_transpose(
                                    CT[m][qi][:, qc * P:(qc + 1) * P],
                                    ctok[:, qc, m * P:(m + 1) * P])

                cnext = [0]  # next global tile whose ctx matmuls are pending

                def cmax(g):
                    # elastic ctx lag: a fresh head's first tiles trail by 5
                    # ticks so its cu-bank WAR on the previous head's
                    # normalize (DVE) is off the PE's critical path
                    return g - (5 if cnext[0] % T_LK < 2 else 1)

                for g, (qh, m, hh, t) in enumerate(ticks):
                    if t == 0:
                        if qh != cur["qh"]:
                            cur["qh"] = qh
                            cur["ctok"] = ctokp.tile(
                                [P, LQ // P // 2, ADC], F16, name="ctok")
                        head_state[g // T_LK] = {
                            "h": 2 * m + hh, "m": m, "hh": hh,
                            "q0": qh * 1024, "gbase": g,
                            "cu": None, "ctok": cur["ctok"],
                        }
                    ro = hh * D
                    q0 = qh * 1024

                    # mandatory: deadlines, pair-0 K chunks, E-ring pressure
                    while work and work[0][0] <= g:
                        run_unit(0)
                    if m == 0 and hh == 0 and qh == 0 and t in (4, 8, 12):
                        emit_kproj(0, t // 4)
                        clk["pe"] = max(clk["pe"],
                                        19100.0 + (t // 4 - 1) * 2900.0) \
                            + PROJ_NS
                    while cnext[0] <= g - ERING + 2:
                        emit_ctx(cnext[0])
                        cnext[0] += 1

                    # paced work: fill the PE up to the point where this
                    # tick's S matmuls still land before exp(g-1) ends.
                    # Also rate-cap per tick: the model clock drifts from
                    # real time, and an uncapped burst of paced work right
                    # before an S matmul starves the exp stream.
                    budget = clk["act"] - LAT - S_NS
                    spent = 0.0
                    nctx = 0
                    progress = True
                    while progress and spent < 800.0:
                        progress = False
                        if nctx < 3 and cnext[0] <= cmax(g) and \
                                clk["pe"] + C_NS <= budget:
                            emit_ctx(cnext[0])
                            cnext[0] += 1
                            spent += C_NS
                            nctx += 1
                            progress = True
                            continue
                        for i in range(len(work)):
                            if work[i][1] <= g and \
                                    clk["pe"] + work[i][2] <= budget:
                                pe0 = clk["pe"]
                                run_unit(i)
                                spent += clk["pe"] - pe0
                                progress = True
                                break

                    # S matmuls for this tick's lk-tile, then exp.
                    # Tick 0 goes in 512-wide halves so the first exp can
                    # start before the second q-chunk's Q projection is done.
                    sw = swp.tile([P, 1024], F32, name="sw")
                    et = epp.tile([P, 1024], F16, name="et")
                    if g == 0:
                        nc.tensor.matmul(
                            sw[:, 0:512], KT[0][0:D, 0:P],
                            QT[0][0:D, 0:512], start=True, stop=True)
                        nc.scalar.activation(et[:, 0:512], sw[:, 0:512],
                                             EXP, scale=SCALE)
                        clk["pe"] += S_NS / 2
                        clk["act"] = clk["pe"] + 1283.0 + 612.0
                        clk["pe"] = max(clk["pe"], 15800.0)
                        emit_qproj(0, 1)
                        clk["pe"] += PROJ_NS
                        nc.tensor.matmul(
                            sw[:, 512:1024], KT[0][0:D, 0:P],
                            QT[0][0:D, 512:1024], start=True, stop=True)
                        nc.scalar.activation(et[:, 512:1024],
                                             sw[:, 512:1024],
                                             EXP, scale=SCALE)
                        clk["pe"] += S_NS / 2
                        clk["act"] = max(clk["act"],
                                         clk["pe"] + LAT) + 612.0
                    else:
                        for nn in range(2):
                            nc.tensor.matmul(
                                sw[:, nn * 512:(nn + 1) * 512],
                                KT[m][ro:ro + D, t * P:(t + 1) * P],
                                QT[m][ro:ro + D,
                                      q0 + nn * 512:q0 + (nn + 1) * 512],
                                start=True, stop=True)
                        nc.scalar.activation(et[:], sw[:], EXP, scale=SCALE)
                        clk["pe"] += S_NS
                        clk["act"] = max(clk["act"], clk["pe"] + LAT) + EXP_NS
                    ering[g % ERING] = et

                # ---------- tail ----------
                while cnext[0] < 256:
                    emit_ctx(cnext[0])
                    cnext[0] += 1
                while work:
                    _, _, _, fn = work.pop(0)
                    fn()
                for d in range(DIM // P):
                    emit_outproj(d, 2)
                    emit_outproj(d, 3)

    nc.compile()
    return nc


def _in_maps(x, embeds, Wq, bq, Wk, bk, Wv, bv, Wo, bo):
    h = np.float16
    f = np.float32
    maps = []
    for c in range(8):
        b, hg = c // 2, c % 2
        s = slice(hg * ADC, (hg + 1) * ADC)
        bo_c = bo if hg == 0 else np.zeros_like(bo)
        maps.append({
            "xT": np.ascontiguousarray(x[b].T, dtype=h),
            "embT": np.ascontiguousarray(embeds[b].T, dtype=h),
            "Wq": np.ascontiguousarray(Wq[:, s], dtype=h),
            "Wk": np.ascontiguousarray(Wk[:, s], dtype=h),
            "Wv": np.ascontiguousarray(Wv[:, s], dtype=h),
            "Wo": np.ascontiguousarray(Wo[s, :], dtype=h),
            "bq": np.ascontiguousarray(bq[s].reshape(M_AD, P).T, dtype=f),
            "bk": np.ascontiguousarray(bk[s].reshape(M_AD, P).T, dtype=f),
            "bvb": np.ascontiguousarray(np.tile(bv[s], (P, 1)), dtype=f),
            "bo": np.ascontiguousarray(
                bo_c.reshape(DIM // P, P).T, dtype=f),
            "ident": np.eye(P, dtype=h),
        })
    return maps


def kernel(x, embeds, Wq, bq, Wk, bk, Wv, bv, Wo, bo, _trace=False,
           _tmpdir=None):
    x = np.asarray(x); embeds = np.asarray(embeds)
    Wq = np.asarray(Wq); bq = np.asarray(bq)
    Wk = np.asarray(Wk); bk = np.asarray(bk)
    Wv = np.asarray(Wv); bv = np.asarray(bv)
    Wo = np.asarray(Wo); bo = np.asarray(bo)

    if "nc" not in _CACHE:
        _CACHE["nc"] = _build()
    nc = _CACHE["nc"]

    maps = _in_maps(x, embeds, Wq, bq, Wk, bk, Wv, bv, Wo, bo)
    res = run_bass_kernel_spmd(nc, maps, core_ids=list(range(8)),
                               trace=_trace, tmpdir=_tmpdir)
    if _trace:
        _CACHE["last_exec_time_ns"] = res.exec_time_ns
    _CACHE["last_results"] = res

    out = np.empty((B, LQ, DIM), np.float32)
    for b in range(B):
        acc = (res.results[2 * b]["outT"].astype(np.float32)
               + res.results[2 * b + 1]["outT"].astype(np.float32))
        out[b] = acc.T
    return out
